# revision 1
# baseline (speedup 1.0000x reference)
import numpy as np
import dec_kernel

_nc_cache = None


def prep_inputs(inputs):
    return dec_kernel.prep_inputs(inputs)


def kernel(**inputs):
    global _nc_cache
    if _nc_cache is None:
        _nc_cache = dec_kernel.build(L=128)
    logits, attw = dec_kernel.run(inputs, L=128, nc=_nc_cache)
    return logits, attw
